# revision 6
# baseline (speedup 1.0000x reference)
"""Mixtral-style MoE (top-2 of 8 experts) on 8 TRN2 NeuronCores.

Strategy (expert-parallel, matching TENSOR_EXPERT_PARALLEL):
  - Host: router (logits -> softmax -> top-2 -> normalized weights), then
    shard: core e receives the tokens routed to expert e (gathered and
    pre-transposed to [H, C]) plus expert e's w1/w3/w2 (bf16, pre-packed
    into PE-friendly [128 x free] tiles).
  - Device (SPMD, identical program on 8 cores): h1T = w1 @ xeT,
    h3T = w3 @ xeT, gT = silu(h1T) * h3T (bf16), then the down-proj in
    output-transposed orientation: outT[h, :] = sum_f w2T-tile @ gT
    (tokens stay on the moving axis, so no padded-partition waste on
    the partial token chunk).  Pure GEMM pipeline; all DMAs linear.
  - Host: scatter-add each core's [H, count_e] contribution (scaled by
    the routing weight, applied host-side) into the [T, H] output.

Compute in bf16 (fp32 PSUM accumulation) keeps the TensorEngine at its
78.6 TF/s peak; fp8 DoubleRow would be ~1.8x faster but its ~3-6%
quantization error blows the 2e-2 correctness budget (measured).
Sparse routing means each core does C = max expert load (~1071)
token-columns instead of all 4096.

PE-time floor at C=1071: phase A 28*2*8*C = 200us + phase B 8*28*C =
100us = 300us @2.4GHz.  Engine init (~6.6us) and first DMA bytes
(~8.3us) are fixed NEFF costs; warmup matmuls bridge them while the
HAM clock ramps.  fp0+fp1 run hk-outer *interleaved* with the token
range split {ci0,ci1} then {ci2} (exactly 8 PSUM banks), stretching
the xe consumption window so three ~130GB/s DMA queues deliver every
chunk just in time -- no stalls, no half-clock dip.  Phase B stages
each 128-row H chunk in a full-width SBUF tile and ships it as one
large-packet DMA, alternating queues, so the output drain collapses
to the final piece plus teardown.
"""

import numpy as np
import ml_dtypes

B, S, H, F, E, TOP_K = 2, 2048, 1024, 3584, 8, 2
N_CORES = 8
P = 128
HK = H // P   # 8 contraction chunks for up-proj
FP = F // P   # 28 partition chunks of the FFN dim
HO = H // P   # 8 output-row chunks of H for the down-proj

BF16 = ml_dtypes.bfloat16

_BUILD_CACHE = {}
LAST_EXEC_TIME_NS = None


def _ensure_axon_hooks_stub():
    """bass_utils imports antenv.axon_hooks when BASS_TRACE is set; the
    agent image lacks it.  Register a None-hook stub so a stray
    BASS_TRACE env var degrades to an untraced run instead of crashing.
    """
    import sys, types

    try:
        import antenv.axon_hooks  # noqa: F401
        return
    except ImportError:
        pass
    mod = types.ModuleType("antenv.axon_hooks")
    mod._hook = None
    mod.set_axon_ntff_profile_hook = lambda h: setattr(mod, "_hook", h)
    mod.get_axon_ntff_profile_hook = lambda: mod._hook
    sys.modules["antenv.axon_hooks"] = mod
    try:
        import antenv

        antenv.axon_hooks = mod
    except ImportError:
        pass


def _chunks(total, maxc):
    """Split `total` into equal-ish chunks <= maxc (PSUM free-dim cap)."""
    n = -(-total // maxc)
    base, rem = divmod(total, n)
    sizes = [base + (1 if i < rem else 0) for i in range(n)]
    out, off = [], 0
    for c in sizes:
        out.append((off, c))
        off += c
    return out


def _build(C):
    """Build + compile the SPMD Bass program for token capacity C."""
    import concourse.bacc as bacc
    import concourse.mybir as mybir
    from concourse.tile import TileContext

    bf = mybir.dt.bfloat16
    f32 = mybir.dt.float32

    nc = bacc.Bacc("TRN2", target_bir_lowering=False, debug=False,
                   num_devices=N_CORES)
    xe = nc.dram_tensor("xe", [HK, P, C], bf, kind="ExternalInput")
    w1p = nc.dram_tensor("w1p", [FP, P, H], bf, kind="ExternalInput")
    w3p = nc.dram_tensor("w3p", [FP, P, H], bf, kind="ExternalInput")
    w2p = nc.dram_tensor("w2p", [FP, P, H], bf, kind="ExternalInput")
    outT = nc.dram_tensor("outT", [HO, P, C], bf, kind="ExternalOutput")

    cn_chunks = _chunks(C, 512)
    NCI = len(cn_chunks)
    # fp0/fp1 warm-start passes: first the leading chunks, then the last
    ci_pass1 = list(range(NCI - 1)) if NCI > 1 else [0]
    ci_pass2 = [NCI - 1] if NCI > 1 else []
    silu = mybir.ActivationFunctionType.Silu
    copy = mybir.ActivationFunctionType.Copy

    with TileContext(nc) as tc:
        with (
            tc.tile_pool(name="persist", bufs=1) as persist,
            tc.tile_pool(name="wload", bufs=2) as wload,
            tc.tile_pool(name="gpool", bufs=1) as gpool,
            tc.tile_pool(name="evac", bufs=4) as evac,
            tc.tile_pool(name="ost", bufs=3) as ost,
            tc.tile_pool(name="psum", bufs=4, space="PSUM") as psum,
        ):
            # HAM warmup: the PE clock-gate needs ~3.4us of sustained
            # activity to lift 1.2 -> 2.4 GHz, and the first DMA bytes
            # only land ~8.3us in (engine init + queue spin-up).  Dummy
            # matmuls bridge the gap; the memset runs on GpSimd.
            warm = persist.tile([P, 512], bf, tag="warm", name="warm")
            nc.gpsimd.memset(warm[:], 0.0)
            wps = psum.tile([P, 512], f32, tag="ps1", name="wps")
            for i in range(6):
                nc.tensor.matmul(wps[:], warm[:, 0:P], warm[:],
                                 start=True, stop=True)

            # Startup DMA schedule.  Three ~130GB/s HWDGE queues
            # (sync/scalar/gpsimd), need-ordered so every operand lands
            # just before the interleaved fp0+fp1 chains (1.19us per hk
            # step) reach it:
            #   gpsimd: xe0a, xe1a, xe2, xe5, then the 28 w2 residents
            #   sync:   w1t0/w1t1 first slices, xe0b, xe1b, mid slices,
            #           xe3, last slices, xe6, then the fp>=2 w1 stream
            #   scalar: same with w3 / xe0c, xe1c, xe4, xe7
            xet = [persist.tile([P, C], bf, tag=f"xe{hk}", name=f"xe{hk}")
                   for hk in range(HK)]
            x3 = _chunks(C, -(-C // 3))  # 3-way column split for xe0/xe1

            def xe_piece(eng, hk, piece):
                off, sz = x3[piece]
                eng.dma_start(out=xet[hk][:, off:off + sz],
                              in_=xe[hk][:, off:off + sz])

            xe_piece(nc.gpsimd, 0, 0)

            w1t0 = wload.tile([P, H], bf, tag="w1")
            w1t1 = wload.tile([P, H], bf, tag="w1")
            w3t0 = wload.tile([P, H], bf, tag="w3")
            w3t1 = wload.tile([P, H], bf, tag="w3")
            WSLICES = [(0, P), (P, 4 * P), (4 * P, H)]
            s0, s1 = WSLICES[0]
            nc.sync.dma_start(out=w1t0[:, s0:s1], in_=w1p[0][:, s0:s1])
            nc.sync.dma_start(out=w1t1[:, s0:s1], in_=w1p[1][:, s0:s1])
            nc.scalar.dma_start(out=w3t0[:, s0:s1], in_=w3p[0][:, s0:s1])
            nc.scalar.dma_start(out=w3t1[:, s0:s1], in_=w3p[1][:, s0:s1])

            xe_piece(nc.sync, 0, 1)
            xe_piece(nc.scalar, 0, 2)
            xe_piece(nc.gpsimd, 1, 0)
            xe_piece(nc.sync, 1, 1)
            xe_piece(nc.scalar, 1, 2)

            s0, s1 = WSLICES[1]
            nc.sync.dma_start(out=w1t0[:, s0:s1], in_=w1p[0][:, s0:s1])
            nc.sync.dma_start(out=w1t1[:, s0:s1], in_=w1p[1][:, s0:s1])
            nc.scalar.dma_start(out=w3t0[:, s0:s1], in_=w3p[0][:, s0:s1])
            nc.scalar.dma_start(out=w3t1[:, s0:s1], in_=w3p[1][:, s0:s1])

            nc.gpsimd.dma_start(out=xet[2][:], in_=xe[2])
            nc.sync.dma_start(out=xet[3][:], in_=xe[3])
            nc.scalar.dma_start(out=xet[4][:], in_=xe[4])

            s0, s1 = WSLICES[2]
            nc.sync.dma_start(out=w1t0[:, s0:s1], in_=w1p[0][:, s0:s1])
            nc.sync.dma_start(out=w1t1[:, s0:s1], in_=w1p[1][:, s0:s1])
            nc.scalar.dma_start(out=w3t0[:, s0:s1], in_=w3p[0][:, s0:s1])
            nc.scalar.dma_start(out=w3t1[:, s0:s1], in_=w3p[1][:, s0:s1])

            nc.gpsimd.dma_start(out=xet[5][:], in_=xe[5])
            nc.sync.dma_start(out=xet[6][:], in_=xe[6])
            nc.scalar.dma_start(out=xet[7][:], in_=xe[7])

            # w2 residents stream on the gpsimd queue during phase A.
            w2t = []
            for fp in range(FP):
                t = persist.tile([P, H], bf, tag=f"w2_{fp}", name=f"w2_{fp}")
                nc.gpsimd.dma_start(out=t[:], in_=w2p[fp])
                w2t.append(t)

            gt = [gpool.tile([P, C], bf, tag=f"g{fp}", name=f"g{fp}")
                  for fp in range(FP)]

            # Phase A: h1T/h3T = w1/w3 @ xeT per 128-row chunk of F,
            # fused SwiGLU into gT (bf16).
            # fp0+fp1 interleave per hk so xe[hk] is consumed at the DMA
            # delivery rate; the token range is split into two passes so
            # the live accumulators fit exactly in the 8 PSUM banks
            # (2 fp x 2 mat x len(ci_pass1) <= 8).
            wts = {(0, 1): w1t0, (0, 3): w3t0, (1, 1): w1t1, (1, 3): w3t1}
            for ci_group in (ci_pass1, ci_pass2):
                if not ci_group:
                    continue
                pss = {}
                for fp in (0, 1):
                    for mat in (1, 3):
                        for ci in ci_group:
                            pss[(fp, mat, ci)] = psum.tile(
                                [P, 512], f32, tag=f"ps{mat}",
                                name=f"ps{mat}_c{ci}_f{fp}",
                            )
                for hk in range(HK):
                    for fp in (0, 1):
                        for mat in (1, 3):
                            for ci in ci_group:
                                coff, csz = cn_chunks[ci]
                                nc.tensor.matmul(
                                    pss[(fp, mat, ci)][:, :csz],
                                    wts[(fp, mat)][:, hk * P:(hk + 1) * P],
                                    xet[hk][:, coff:coff + csz],
                                    start=(hk == 0), stop=(hk == HK - 1),
                                )
                for fp in (0, 1):
                    for ci in ci_group:
                        coff, csz = cn_chunks[ci]
                        sil = evac.tile([P, 512], f32, tag="sil",
                                        name=f"sil_f{fp}_{ci}")
                        nc.scalar.activation(
                            sil[:, :csz], pss[(fp, 1, ci)][:, :csz], silu)
                        nc.vector.tensor_mul(
                            gt[fp][:, coff:coff + csz], sil[:, :csz],
                            pss[(fp, 3, ci)][:, :csz],
                        )

            for fp in range(2, FP):
                w1t = wload.tile([P, H], bf, tag="w1")
                nc.sync.dma_start(out=w1t[:], in_=w1p[fp])
                w3t = wload.tile([P, H], bf, tag="w3")
                nc.scalar.dma_start(out=w3t[:], in_=w3p[fp])
                for (coff, csz) in cn_chunks:
                    ps1 = psum.tile([P, 512], f32, tag="ps1")
                    ps3 = psum.tile([P, 512], f32, tag="ps3")
                    for hk in range(HK):
                        nc.tensor.matmul(
                            ps1[:, :csz],
                            w1t[:, hk * P:(hk + 1) * P],
                            xet[hk][:, coff:coff + csz],
                            start=(hk == 0), stop=(hk == HK - 1),
                        )
                    for hk in range(HK):
                        nc.tensor.matmul(
                            ps3[:, :csz],
                            w3t[:, hk * P:(hk + 1) * P],
                            xet[hk][:, coff:coff + csz],
                            start=(hk == 0), stop=(hk == HK - 1),
                        )
                    sil = evac.tile([P, 512], f32, tag="sil")
                    nc.scalar.activation(sil[:, :csz], ps1[:, :csz], silu)
                    nc.vector.tensor_mul(
                        gt[fp][:, coff:coff + csz], sil[:, :csz], ps3[:, :csz]
                    )

            # Phase B: outT[h] chunk [128 H-rows, csz tokens] =
            # sum_fp w2T-tile[fp,h] @ gT[fp].  Tokens ride the moving
            # axis, so the partial token chunk costs only its true
            # column count.  Each h stages into one full-width tile and
            # ships as a single large-packet DMA; routing weights are
            # applied host-side.  Shares the phase-A PSUM pool (no
            # pool-transition barrier).
            for h in range(HO):
                oh = ost.tile([P, C], bf, tag="o", name=f"o{h}")
                for ci, (coff, csz) in enumerate(cn_chunks):
                    pb = psum.tile([P, 512], f32,
                                   tag="ps1" if (h * NCI + ci) % 2 == 0
                                   else "ps3")
                    for fp in range(FP):
                        nc.tensor.matmul(
                            pb[:, :csz],
                            w2t[fp][:, h * P:(h + 1) * P],
                            gt[fp][:, coff:coff + csz],
                            start=(fp == 0), stop=(fp == FP - 1),
                        )
                    nc.scalar.activation(oh[:, coff:coff + csz],
                                         pb[:, :csz], copy)
                if h == HO - 1:
                    # final piece: split across both queues so the
                    # end-of-kernel drain is half as long
                    half = C // 2
                    nc.sync.dma_start(out=outT[h][:, 0:half],
                                      in_=oh[:, 0:half])
                    nc.scalar.dma_start(out=outT[h][:, half:C],
                                        in_=oh[:, half:C])
                else:
                    e = nc.sync if h % 2 == 0 else nc.scalar
                    e.dma_start(out=outT[h], in_=oh[:])

    nc.compile()
    return nc


def kernel(hidden_states, gate_w, w1, w2, w3, _trace=False):
    global LAST_EXEC_TIME_NS
    _ensure_axon_hooks_stub()
    from concourse.bass_utils import run_bass_kernel_spmd

    x = np.asarray(hidden_states, dtype=np.float32).reshape(-1, H)
    gate_w = np.asarray(gate_w, dtype=np.float32)
    w1 = np.asarray(w1, dtype=np.float32)
    w2 = np.asarray(w2, dtype=np.float32)
    w3 = np.asarray(w3, dtype=np.float32)
    T = x.shape[0]

    # Router (f32, same math as the module): softmax over experts, top-2,
    # renormalized weights.
    logits = x @ gate_w.T
    p = np.exp(logits - logits.max(-1, keepdims=True))
    p /= p.sum(-1, keepdims=True)
    sel = np.argpartition(-p, TOP_K - 1, axis=-1)[:, :TOP_K]
    rw = np.take_along_axis(p, sel, axis=-1)
    rw = rw / rw.sum(-1, keepdims=True)

    idx_e, cv_e = [], []
    for e in range(E):
        hit = sel == e                      # [T, K]
        idx = np.nonzero(hit.any(axis=1))[0]
        w = np.where(hit[idx, 0], rw[idx, 0], rw[idx, 1])
        idx_e.append(idx)
        cv_e.append(w.astype(np.float32))

    # SBUF budget (xe + gT residents) caps the per-run token capacity.
    # Actual data peaks at cmax ~1071; the segment loop only engages for
    # pathologically imbalanced routing.
    CMAX_HW = 1344
    cmax = max(len(i) for i in idx_e)
    n_seg = max(1, -(-cmax // CMAX_HW))
    seg_idx = [np.array_split(idx_e[e], n_seg) for e in range(E)]
    seg_cv = [np.array_split(cv_e[e], n_seg) for e in range(E)]
    C = max(512, max(len(s) for parts in seg_idx for s in parts))

    if C not in _BUILD_CACHE:
        _BUILD_CACHE[C] = _build(C)
    nc = _BUILD_CACHE[C]

    x_bf = x.astype(BF16)
    w_packed = []
    for e in range(E):
        w1pk = np.ascontiguousarray(
            w1[e].astype(BF16).reshape(FP, P, HK, P).transpose(0, 3, 2, 1)
        ).reshape(FP, P, H)
        w3pk = np.ascontiguousarray(
            w3[e].astype(BF16).reshape(FP, P, HK, P).transpose(0, 3, 2, 1)
        ).reshape(FP, P, H)
        w2pk = np.ascontiguousarray(w2[e].T.astype(BF16)).reshape(FP, P, H)
        w_packed.append((w1pk, w3pk, w2pk))

    out = np.zeros((T, H), dtype=np.float32)
    LAST_EXEC_TIME_NS = None
    for seg in range(n_seg):
        in_maps = []
        for e in range(E):
            idx = seg_idx[e][seg]
            n = len(idx)
            xeT = np.zeros((H, C), dtype=BF16)
            xeT[:, :n] = x_bf[idx].T
            w1pk, w3pk, w2pk = w_packed[e]
            in_maps.append({
                "xe": np.ascontiguousarray(xeT.reshape(HK, P, C)),
                "w1p": w1pk,
                "w3p": w3pk,
                "w2p": w2pk,
            })
        res = run_bass_kernel_spmd(
            nc, in_maps, core_ids=list(range(N_CORES)), trace=_trace
        )
        if res.exec_time_ns is not None:
            LAST_EXEC_TIME_NS = (LAST_EXEC_TIME_NS or 0) + res.exec_time_ns
        for e in range(E):
            idx = seg_idx[e][seg]
            n = len(idx)
            if n:
                oT = np.asarray(res.results[e]["outT"],
                                dtype=np.float32).reshape(H, C)
                out[idx] += oT[:, :n].T * seg_cv[e][seg][:, None]
    return out.reshape(B, S, H)


# revision 9
# speedup vs baseline: 1.0431x; 1.0431x over previous
"""Mixtral-style MoE (top-2 of 8 experts) on 8 TRN2 NeuronCores.

Strategy (expert-parallel, matching TENSOR_EXPERT_PARALLEL):
  - Host: router (logits -> softmax -> top-2 -> normalized weights), then
    shard: core e receives the tokens routed to expert e (gathered and
    pre-transposed to [H, C]) plus expert e's w1/w3/w2 (bf16, pre-packed
    into PE-friendly [128 x free] tiles).
  - Device (SPMD, identical program on 8 cores): h1T = w1 @ xeT,
    h3T = w3 @ xeT, gT = silu(h1T) * h3T (bf16), then the down-proj in
    output-transposed orientation: outT[h, :] = sum_f w2T-tile @ gT
    (tokens stay on the moving axis, so no padded-partition waste on
    the partial token chunk).  Pure GEMM pipeline; all DMAs linear.
  - Host: scatter-add each core's [H, count_e] contribution (scaled by
    the routing weight, applied host-side) into the [T, H] output.

Compute in bf16 (fp32 PSUM accumulation) keeps the TensorEngine at its
78.6 TF/s peak; fp8 DoubleRow would be ~1.8x faster but its ~3-6%
quantization error blows the 2e-2 correctness budget (measured).
Sparse routing means each core does C = max expert load (~1071)
token-columns instead of all 4096.

PE-time floor at C=1071: phase A 28*2*8*C = 200us + phase B 8*28*C =
100us = 300us @2.4GHz.  Engine init (~6.6us) and first DMA bytes
(~8.3us) are fixed NEFF costs; warmup matmuls bridge them while the
HAM clock ramps.  fp0+fp1 run hk-outer *interleaved* with the token
range split {ci0,ci1} then {ci2} (exactly 8 PSUM banks), stretching
the xe consumption window so three ~130GB/s DMA queues deliver every
chunk just in time -- no stalls, no half-clock dip.  Phase B stages
each 128-row H chunk in a full-width SBUF tile and ships it as one
large-packet DMA, alternating queues, so the output drain collapses
to the final piece plus teardown.
"""

import numpy as np
import ml_dtypes

B, S, H, F, E, TOP_K = 2, 2048, 1024, 3584, 8, 2
N_CORES = 8
P = 128
HK = H // P   # 8 contraction chunks for up-proj
FP = F // P   # 28 partition chunks of the FFN dim
HO = H // P   # 8 output-row chunks of H for the down-proj

BF16 = ml_dtypes.bfloat16

_BUILD_CACHE = {}
LAST_EXEC_TIME_NS = None


def _ensure_axon_hooks_stub():
    """bass_utils imports antenv.axon_hooks when BASS_TRACE is set; the
    agent image lacks it.  Register a None-hook stub so a stray
    BASS_TRACE env var degrades to an untraced run instead of crashing.
    """
    import sys, types

    try:
        import antenv.axon_hooks  # noqa: F401
        return
    except ImportError:
        pass
    mod = types.ModuleType("antenv.axon_hooks")
    mod._hook = None
    mod.set_axon_ntff_profile_hook = lambda h: setattr(mod, "_hook", h)
    mod.get_axon_ntff_profile_hook = lambda: mod._hook
    sys.modules["antenv.axon_hooks"] = mod
    try:
        import antenv

        antenv.axon_hooks = mod
    except ImportError:
        pass


def _chunks(total, maxc):
    """Split `total` into equal-ish chunks <= maxc (PSUM free-dim cap)."""
    n = -(-total // maxc)
    base, rem = divmod(total, n)
    sizes = [base + (1 if i < rem else 0) for i in range(n)]
    out, off = [], 0
    for c in sizes:
        out.append((off, c))
        off += c
    return out


def _build(C):
    """Build + compile the SPMD Bass program for token capacity C."""
    import concourse.bacc as bacc
    import concourse.mybir as mybir
    from concourse.tile import TileContext

    bf = mybir.dt.bfloat16
    f32 = mybir.dt.float32

    nc = bacc.Bacc("TRN2", target_bir_lowering=False, debug=False,
                   num_devices=N_CORES)
    # xe is packed [P, HK*C]: per-partition rows hold all HK contraction
    # chunks contiguously, so a multi-chunk column range is ONE wide DMA
    # (queues are descriptor-rate limited: any [128, w] piece costs
    # ~2.1us regardless of w, so fewer/wider pieces win).
    xe = nc.dram_tensor("xe", [P, HK * C], bf, kind="ExternalInput")
    w1p = nc.dram_tensor("w1p", [FP, P, H], bf, kind="ExternalInput")
    w3p = nc.dram_tensor("w3p", [FP, P, H], bf, kind="ExternalInput")
    w2p = nc.dram_tensor("w2p", [FP, P, H], bf, kind="ExternalInput")
    outT = nc.dram_tensor("outT", [HO, P, C], bf, kind="ExternalOutput")

    cn_chunks = _chunks(C, 512)
    NCI = len(cn_chunks)
    silu = mybir.ActivationFunctionType.Silu
    copy = mybir.ActivationFunctionType.Copy

    with TileContext(nc) as tc:
        with (
            tc.tile_pool(name="persist", bufs=1) as persist,
            tc.tile_pool(name="wload", bufs=3) as wload,
            tc.tile_pool(name="gpool", bufs=1) as gpool,
            tc.tile_pool(name="evac", bufs=4) as evac,
            tc.tile_pool(name="ost", bufs=3) as ost,
            tc.tile_pool(name="psum", bufs=4, space="PSUM") as psum,
        ):
            # HAM warmup: the PE clock-gate needs ~3.4us of sustained
            # activity to lift 1.2 -> 2.4 GHz, and the first DMA bytes
            # only land ~8.3us in (engine init + queue spin-up).  Dummy
            # matmuls bridge the gap; the memset runs on GpSimd.
            warm = persist.tile([P, 512], bf, tag="warm", name="warm")
            nc.gpsimd.memset(warm[:], 0.0)
            wps = psum.tile([P, 512], f32, tag="ps1", name="wps")
            for i in range(8):
                nc.tensor.matmul(wps[:], warm[:, 0:P], warm[:],
                                 start=True, stop=True)

            # Startup DMA schedule: one wide piece per queue per round.
            # Round 1 (~2.1us each): w1t0 | w3t0 | xe[hk0-1] -- the fp0
            # chain's first operands.  Round 2: xe[hk2-3] | xe[hk4-5] |
            # xe[hk6-7].  Round 3: w1t1 | w3t1 | w2 stream.  The fp0/fp1
            # hk-outer chains consume xe at ~0.9us/chunk starting ~11us,
            # so every piece lands just ahead of use.
            xet = persist.tile([P, HK * C], bf, tag="xe", name="xet")

            w1t0 = wload.tile([P, H], bf, tag="w1")
            nc.sync.dma_start(out=w1t0[:], in_=w1p[0])
            w3t0 = wload.tile([P, H], bf, tag="w3")
            nc.scalar.dma_start(out=w3t0[:], in_=w3p[0])
            nc.gpsimd.dma_start(out=xet[:, 0:2 * C], in_=xe[:, 0:2 * C])

            nc.sync.dma_start(out=xet[:, 2 * C:4 * C], in_=xe[:, 2 * C:4 * C])
            nc.scalar.dma_start(out=xet[:, 4 * C:6 * C],
                                in_=xe[:, 4 * C:6 * C])
            nc.gpsimd.dma_start(out=xet[:, 6 * C:8 * C],
                                in_=xe[:, 6 * C:8 * C])

            w1t1 = wload.tile([P, H], bf, tag="w1")
            nc.sync.dma_start(out=w1t1[:], in_=w1p[1])
            w3t1 = wload.tile([P, H], bf, tag="w3")
            nc.scalar.dma_start(out=w3t1[:], in_=w3p[1])

            # w2 residents stream on the gpsimd queue during phase A.
            w2t = []
            for fp in range(FP):
                t = persist.tile([P, H], bf, tag=f"w2_{fp}", name=f"w2_{fp}")
                nc.gpsimd.dma_start(out=t[:], in_=w2p[fp])
                w2t.append(t)

            gt = [gpool.tile([P, C], bf, tag=f"g{fp}", name=f"g{fp}")
                  for fp in range(FP)]

            # Phase A: h1T/h3T = w1/w3 @ xeT per 128-row chunk of F,
            # fused SwiGLU into gT (bf16).
            for fp in range(FP):
                if fp < 2:
                    # hk-outer: each matmul chain consumes xe[hk] as it
                    # lands instead of stalling on the whole activation
                    # load before the first instruction.  Two chains
                    # (~15us PE) cover the startup DMA window.
                    w1t, w3t = (w1t0, w3t0) if fp == 0 else (w1t1, w3t1)
                    pss = {}
                    for mat in (1, 3):
                        for ci in range(NCI):
                            pss[(mat, ci)] = psum.tile(
                                [P, 512], f32, tag=f"ps{mat}",
                                name=f"ps{mat}_c{ci}_f{fp}",
                            )
                    for hk in range(HK):
                        for mat, wt in ((1, w1t), (3, w3t)):
                            for ci, (coff, csz) in enumerate(cn_chunks):
                                nc.tensor.matmul(
                                    pss[(mat, ci)][:, :csz],
                                    wt[:, hk * P:(hk + 1) * P],
                                    xet[:, hk * C + coff:hk * C + coff + csz],
                                    start=(hk == 0), stop=(hk == HK - 1),
                                )
                    for ci, (coff, csz) in enumerate(cn_chunks):
                        sil = evac.tile([P, 512], f32, tag="sil",
                                        name=f"sil_f{fp}_{ci}")
                        nc.scalar.activation(
                            sil[:, :csz], pss[(1, ci)][:, :csz], silu)
                        nc.vector.tensor_mul(
                            gt[fp][:, coff:coff + csz], sil[:, :csz],
                            pss[(3, ci)][:, :csz],
                        )
                    continue
                w1t = wload.tile([P, H], bf, tag="w1")
                nc.sync.dma_start(out=w1t[:], in_=w1p[fp])
                w3t = wload.tile([P, H], bf, tag="w3")
                nc.scalar.dma_start(out=w3t[:], in_=w3p[fp])
                for (coff, csz) in cn_chunks:
                    ps1 = psum.tile([P, 512], f32, tag="ps1")
                    ps3 = psum.tile([P, 512], f32, tag="ps3")
                    for hk in range(HK):
                        nc.tensor.matmul(
                            ps1[:, :csz],
                            w1t[:, hk * P:(hk + 1) * P],
                            xet[:, hk * C + coff:hk * C + coff + csz],
                            start=(hk == 0), stop=(hk == HK - 1),
                        )
                    for hk in range(HK):
                        nc.tensor.matmul(
                            ps3[:, :csz],
                            w3t[:, hk * P:(hk + 1) * P],
                            xet[:, hk * C + coff:hk * C + coff + csz],
                            start=(hk == 0), stop=(hk == HK - 1),
                        )
                    sil = evac.tile([P, 512], f32, tag="sil")
                    nc.scalar.activation(sil[:, :csz], ps1[:, :csz], silu)
                    nc.vector.tensor_mul(
                        gt[fp][:, coff:coff + csz], sil[:, :csz], ps3[:, :csz]
                    )

            # Phase B: outT[h] chunk [128 H-rows, csz tokens] =
            # sum_fp w2T-tile[fp,h] @ gT[fp].  Tokens ride the moving
            # axis, so the partial token chunk costs only its true
            # column count.  Each h stages into one full-width tile and
            # ships as a single large-packet DMA; routing weights are
            # applied host-side.  Shares the phase-A PSUM pool (no
            # pool-transition barrier).
            for h in range(HO):
                oh = ost.tile([P, C], bf, tag="o", name=f"o{h}")
                for ci, (coff, csz) in enumerate(cn_chunks):
                    pb = psum.tile([P, 512], f32,
                                   tag="ps1" if (h * NCI + ci) % 2 == 0
                                   else "ps3")
                    for fp in range(FP):
                        nc.tensor.matmul(
                            pb[:, :csz],
                            w2t[fp][:, h * P:(h + 1) * P],
                            gt[fp][:, coff:coff + csz],
                            start=(fp == 0), stop=(fp == FP - 1),
                        )
                    nc.scalar.activation(oh[:, coff:coff + csz],
                                         pb[:, :csz], copy)
                if h == HO - 1:
                    # final piece: split across both queues so the
                    # end-of-kernel drain is half as long
                    half = C // 2
                    nc.sync.dma_start(out=outT[h][:, 0:half],
                                      in_=oh[:, 0:half])
                    nc.scalar.dma_start(out=outT[h][:, half:C],
                                        in_=oh[:, half:C])
                else:
                    e = nc.sync if h % 2 == 0 else nc.scalar
                    e.dma_start(out=outT[h], in_=oh[:])

    nc.compile()
    return nc


def kernel(hidden_states, gate_w, w1, w2, w3, _trace=False):
    global LAST_EXEC_TIME_NS
    _ensure_axon_hooks_stub()
    from concourse.bass_utils import run_bass_kernel_spmd

    x = np.asarray(hidden_states, dtype=np.float32).reshape(-1, H)
    gate_w = np.asarray(gate_w, dtype=np.float32)
    w1 = np.asarray(w1, dtype=np.float32)
    w2 = np.asarray(w2, dtype=np.float32)
    w3 = np.asarray(w3, dtype=np.float32)
    T = x.shape[0]

    # Router (f32, same math as the module): softmax over experts, top-2,
    # renormalized weights.
    logits = x @ gate_w.T
    p = np.exp(logits - logits.max(-1, keepdims=True))
    p /= p.sum(-1, keepdims=True)
    sel = np.argpartition(-p, TOP_K - 1, axis=-1)[:, :TOP_K]
    rw = np.take_along_axis(p, sel, axis=-1)
    rw = rw / rw.sum(-1, keepdims=True)

    idx_e, cv_e = [], []
    for e in range(E):
        hit = sel == e                      # [T, K]
        idx = np.nonzero(hit.any(axis=1))[0]
        w = np.where(hit[idx, 0], rw[idx, 0], rw[idx, 1])
        idx_e.append(idx)
        cv_e.append(w.astype(np.float32))

    # SBUF budget (xe + gT residents) caps the per-run token capacity.
    # Actual data peaks at cmax ~1071; the segment loop only engages for
    # pathologically imbalanced routing.
    CMAX_HW = 1344
    cmax = max(len(i) for i in idx_e)
    n_seg = max(1, -(-cmax // CMAX_HW))
    seg_idx = [np.array_split(idx_e[e], n_seg) for e in range(E)]
    seg_cv = [np.array_split(cv_e[e], n_seg) for e in range(E)]
    C = max(512, max(len(s) for parts in seg_idx for s in parts))

    if C not in _BUILD_CACHE:
        _BUILD_CACHE[C] = _build(C)
    nc = _BUILD_CACHE[C]

    x_bf = x.astype(BF16)
    w_packed = []
    for e in range(E):
        w1pk = np.ascontiguousarray(
            w1[e].astype(BF16).reshape(FP, P, HK, P).transpose(0, 3, 2, 1)
        ).reshape(FP, P, H)
        w3pk = np.ascontiguousarray(
            w3[e].astype(BF16).reshape(FP, P, HK, P).transpose(0, 3, 2, 1)
        ).reshape(FP, P, H)
        w2pk = np.ascontiguousarray(w2[e].T.astype(BF16)).reshape(FP, P, H)
        w_packed.append((w1pk, w3pk, w2pk))

    out = np.zeros((T, H), dtype=np.float32)
    LAST_EXEC_TIME_NS = None
    for seg in range(n_seg):
        in_maps = []
        for e in range(E):
            idx = seg_idx[e][seg]
            n = len(idx)
            xeT = np.zeros((H, C), dtype=BF16)
            xeT[:, :n] = x_bf[idx].T
            w1pk, w3pk, w2pk = w_packed[e]
            # pack [P, HK*C]: partition-major rows holding all HK chunks
            xpk = np.ascontiguousarray(
                xeT.reshape(HK, P, C).transpose(1, 0, 2).reshape(P, HK * C))
            in_maps.append({
                "xe": xpk,
                "w1p": w1pk,
                "w3p": w3pk,
                "w2p": w2pk,
            })
        res = run_bass_kernel_spmd(
            nc, in_maps, core_ids=list(range(N_CORES)), trace=_trace
        )
        if res.exec_time_ns is not None:
            LAST_EXEC_TIME_NS = (LAST_EXEC_TIME_NS or 0) + res.exec_time_ns
        for e in range(E):
            idx = seg_idx[e][seg]
            n = len(idx)
            if n:
                oT = np.asarray(res.results[e]["outT"],
                                dtype=np.float32).reshape(H, C)
                out[idx] += oT[:, :n].T * seg_cv[e][seg][:, None]
    return out.reshape(B, S, H)


# revision 11
# speedup vs baseline: 1.0480x; 1.0047x over previous
"""Mixtral-style MoE (top-2 of 8 experts) on 8 TRN2 NeuronCores.

Strategy (expert-parallel, matching TENSOR_EXPERT_PARALLEL):
  - Host: router (logits -> softmax -> top-2 -> normalized weights), then
    shard: core e receives the tokens routed to expert e (gathered and
    pre-transposed to [H, C]) plus expert e's w1/w3/w2 (bf16, pre-packed
    into PE-friendly [128 x free] tiles).
  - Device (SPMD, identical program on 8 cores): h1T = w1 @ xeT,
    h3T = w3 @ xeT, gT = silu(h1T) * h3T (bf16), then the down-proj in
    output-transposed orientation: outT[h, :] = sum_f w2T-tile @ gT
    (tokens stay on the moving axis, so no padded-partition waste on
    the partial token chunk).  Pure GEMM pipeline; all DMAs linear.
  - Host: scatter-add each core's [H, count_e] contribution (scaled by
    the routing weight, applied host-side) into the [T, H] output.

Compute in bf16 (fp32 PSUM accumulation) keeps the TensorEngine at its
78.6 TF/s peak; fp8 DoubleRow would be ~1.8x faster but its ~3-6%
quantization error blows the 2e-2 correctness budget (measured).
Sparse routing means each core does C = max expert load (~1071)
token-columns instead of all 4096.

PE-time floor at C=1071: phase A 28*2*8*C = 200us + phase B 8*28*C =
100us = 300us @2.4GHz.  Engine init (~6.6us) and first DMA bytes
(~8.3us) are fixed NEFF costs; warmup matmuls bridge them while the
HAM clock ramps.  fp0+fp1 run hk-outer *interleaved* with the token
range split {ci0,ci1} then {ci2} (exactly 8 PSUM banks), stretching
the xe consumption window so three ~130GB/s DMA queues deliver every
chunk just in time -- no stalls, no half-clock dip.  Phase B stages
each 128-row H chunk in a full-width SBUF tile and ships it as one
large-packet DMA, alternating queues, so the output drain collapses
to the final piece plus teardown.
"""

import numpy as np
import ml_dtypes

B, S, H, F, E, TOP_K = 2, 2048, 1024, 3584, 8, 2
N_CORES = 8
P = 128
HK = H // P   # 8 contraction chunks for up-proj
FP = F // P   # 28 partition chunks of the FFN dim
HO = H // P   # 8 output-row chunks of H for the down-proj

BF16 = ml_dtypes.bfloat16

_BUILD_CACHE = {}
LAST_EXEC_TIME_NS = None


def _ensure_axon_hooks_stub():
    """bass_utils imports antenv.axon_hooks when BASS_TRACE is set; the
    agent image lacks it.  Register a None-hook stub so a stray
    BASS_TRACE env var degrades to an untraced run instead of crashing.
    """
    import sys, types

    try:
        import antenv.axon_hooks  # noqa: F401
        return
    except ImportError:
        pass
    mod = types.ModuleType("antenv.axon_hooks")
    mod._hook = None
    mod.set_axon_ntff_profile_hook = lambda h: setattr(mod, "_hook", h)
    mod.get_axon_ntff_profile_hook = lambda: mod._hook
    sys.modules["antenv.axon_hooks"] = mod
    try:
        import antenv

        antenv.axon_hooks = mod
    except ImportError:
        pass


def _chunks(total, maxc):
    """Split `total` into equal-ish chunks <= maxc (PSUM free-dim cap)."""
    n = -(-total // maxc)
    base, rem = divmod(total, n)
    sizes = [base + (1 if i < rem else 0) for i in range(n)]
    out, off = [], 0
    for c in sizes:
        out.append((off, c))
        off += c
    return out


def _build(C):
    """Build + compile the SPMD Bass program for token capacity C."""
    import concourse.bacc as bacc
    import concourse.mybir as mybir
    from concourse.tile import TileContext

    bf = mybir.dt.bfloat16
    f32 = mybir.dt.float32

    nc = bacc.Bacc("TRN2", target_bir_lowering=False, debug=False,
                   num_devices=N_CORES)
    # xe is packed [P, HK*C]: per-partition rows hold all HK contraction
    # chunks contiguously, so a multi-chunk column range is ONE wide DMA
    # (queues are descriptor-rate limited: any [128, w] piece costs
    # ~2.1us regardless of w, so fewer/wider pieces win).
    xe = nc.dram_tensor("xe", [P, HK * C], bf, kind="ExternalInput")
    w1p = nc.dram_tensor("w1p", [FP, P, H], bf, kind="ExternalInput")
    w3p = nc.dram_tensor("w3p", [FP, P, H], bf, kind="ExternalInput")
    w2p = nc.dram_tensor("w2p", [FP, P, H], bf, kind="ExternalInput")
    outT = nc.dram_tensor("outT", [HO, P, C], bf, kind="ExternalOutput")

    cn_chunks = _chunks(C, 512)
    NCI = len(cn_chunks)
    silu = mybir.ActivationFunctionType.Silu
    copy = mybir.ActivationFunctionType.Copy

    with TileContext(nc) as tc:
        with (
            tc.tile_pool(name="persist", bufs=1) as persist,
            tc.tile_pool(name="wload", bufs=3) as wload,
            tc.tile_pool(name="gpool", bufs=1) as gpool,
            tc.tile_pool(name="evac", bufs=4) as evac,
            tc.tile_pool(name="ost", bufs=3) as ost,
            tc.tile_pool(name="psum", bufs=4, space="PSUM") as psum,
        ):
            # HAM warmup: the PE clock-gate needs ~3.4us of sustained
            # activity to lift 1.2 -> 2.4 GHz, and the first DMA bytes
            # only land ~8.3us in (engine init + queue spin-up).  Dummy
            # matmuls bridge the gap; the memset runs on GpSimd.
            warm = persist.tile([P, 512], bf, tag="warm", name="warm")
            nc.gpsimd.memset(warm[:], 0.0)
            # 20 dummies: the first ~8 run at the 1.2GHz p-state (3.4us,
            # lifting the clock at ~11us), the rest at full rate, ending
            # ~13.6us -- right when the round-1/2 DMA pieces land.  The
            # PE never idles pre-ramp, so the HAM gate stays up.
            wps = psum.tile([P, 512], f32, tag="ps1", name="wps")
            for i in range(20):
                nc.tensor.matmul(wps[:], warm[:, 0:P], warm[:],
                                 start=True, stop=True)

            # Startup DMA schedule: one wide piece per queue per round.
            # Round 1 (~2.1us each): w1t0 | w3t0 | xe[hk0-1] -- the fp0
            # chain's first operands.  Round 2: xe[hk2-3] | xe[hk4-5] |
            # xe[hk6-7].  Round 3: w1t1 | w3t1 | w2 stream.  The fp0/fp1
            # hk-outer chains consume xe at ~0.9us/chunk starting ~11us,
            # so every piece lands just ahead of use.
            xet = persist.tile([P, HK * C], bf, tag="xe", name="xet")

            w1t0 = wload.tile([P, H], bf, tag="w1")
            nc.sync.dma_start(out=w1t0[:], in_=w1p[0])
            w3t0 = wload.tile([P, H], bf, tag="w3")
            nc.scalar.dma_start(out=w3t0[:], in_=w3p[0])
            nc.gpsimd.dma_start(out=xet[:, 0:2 * C], in_=xe[:, 0:2 * C])

            nc.sync.dma_start(out=xet[:, 2 * C:4 * C], in_=xe[:, 2 * C:4 * C])
            nc.scalar.dma_start(out=xet[:, 4 * C:6 * C],
                                in_=xe[:, 4 * C:6 * C])
            nc.gpsimd.dma_start(out=xet[:, 6 * C:8 * C],
                                in_=xe[:, 6 * C:8 * C])

            w1t1 = wload.tile([P, H], bf, tag="w1")
            nc.sync.dma_start(out=w1t1[:], in_=w1p[1])
            w3t1 = wload.tile([P, H], bf, tag="w3")
            nc.scalar.dma_start(out=w3t1[:], in_=w3p[1])

            # w2 residents stream on the gpsimd queue during phase A.
            w2t = []
            for fp in range(FP):
                t = persist.tile([P, H], bf, tag=f"w2_{fp}", name=f"w2_{fp}")
                nc.gpsimd.dma_start(out=t[:], in_=w2p[fp])
                w2t.append(t)

            gt = [gpool.tile([P, C], bf, tag=f"g{fp}", name=f"g{fp}")
                  for fp in range(FP)]

            # Phase A: h1T/h3T = w1/w3 @ xeT per 128-row chunk of F,
            # fused SwiGLU into gT (bf16).
            for fp in range(FP):
                if fp < 2:
                    # hk-outer: each matmul chain consumes xe[hk] as it
                    # lands instead of stalling on the whole activation
                    # load before the first instruction.  Two chains
                    # (~15us PE) cover the startup DMA window.
                    w1t, w3t = (w1t0, w3t0) if fp == 0 else (w1t1, w3t1)
                    pss = {}
                    for mat in (1, 3):
                        for ci in range(NCI):
                            pss[(mat, ci)] = psum.tile(
                                [P, 512], f32, tag=f"ps{mat}",
                                name=f"ps{mat}_c{ci}_f{fp}",
                            )
                    for hk in range(HK):
                        for mat, wt in ((1, w1t), (3, w3t)):
                            for ci, (coff, csz) in enumerate(cn_chunks):
                                nc.tensor.matmul(
                                    pss[(mat, ci)][:, :csz],
                                    wt[:, hk * P:(hk + 1) * P],
                                    xet[:, hk * C + coff:hk * C + coff + csz],
                                    start=(hk == 0), stop=(hk == HK - 1),
                                )
                    for ci, (coff, csz) in enumerate(cn_chunks):
                        sil = evac.tile([P, 512], f32, tag="sil",
                                        name=f"sil_f{fp}_{ci}")
                        nc.scalar.activation(
                            sil[:, :csz], pss[(1, ci)][:, :csz], silu)
                        nc.vector.tensor_mul(
                            gt[fp][:, coff:coff + csz], sil[:, :csz],
                            pss[(3, ci)][:, :csz],
                        )
                    continue
                w1t = wload.tile([P, H], bf, tag="w1")
                nc.sync.dma_start(out=w1t[:], in_=w1p[fp])
                w3t = wload.tile([P, H], bf, tag="w3")
                nc.scalar.dma_start(out=w3t[:], in_=w3p[fp])
                for (coff, csz) in cn_chunks:
                    ps1 = psum.tile([P, 512], f32, tag="ps1")
                    ps3 = psum.tile([P, 512], f32, tag="ps3")
                    for hk in range(HK):
                        nc.tensor.matmul(
                            ps1[:, :csz],
                            w1t[:, hk * P:(hk + 1) * P],
                            xet[:, hk * C + coff:hk * C + coff + csz],
                            start=(hk == 0), stop=(hk == HK - 1),
                        )
                    for hk in range(HK):
                        nc.tensor.matmul(
                            ps3[:, :csz],
                            w3t[:, hk * P:(hk + 1) * P],
                            xet[:, hk * C + coff:hk * C + coff + csz],
                            start=(hk == 0), stop=(hk == HK - 1),
                        )
                    sil = evac.tile([P, 512], f32, tag="sil")
                    nc.scalar.activation(sil[:, :csz], ps1[:, :csz], silu)
                    nc.vector.tensor_mul(
                        gt[fp][:, coff:coff + csz], sil[:, :csz], ps3[:, :csz]
                    )

            # Phase B: outT[h] chunk [128 H-rows, csz tokens] =
            # sum_fp w2T-tile[fp,h] @ gT[fp].  Tokens ride the moving
            # axis, so the partial token chunk costs only its true
            # column count.  Each h stages into one full-width tile and
            # ships as a single large-packet DMA; routing weights are
            # applied host-side.  Shares the phase-A PSUM pool (no
            # pool-transition barrier).
            for h in range(HO):
                oh = ost.tile([P, C], bf, tag="o", name=f"o{h}")
                last_h = h == HO - 1
                for ci, (coff, csz) in enumerate(cn_chunks):
                    pb = psum.tile([P, 512], f32,
                                   tag="ps1" if (h * NCI + ci) % 2 == 0
                                   else "ps3")
                    for fp in range(FP):
                        nc.tensor.matmul(
                            pb[:, :csz],
                            w2t[fp][:, h * P:(h + 1) * P],
                            gt[fp][:, coff:coff + csz],
                            start=(fp == 0), stop=(fp == FP - 1),
                        )
                    nc.scalar.activation(oh[:, coff:coff + csz],
                                         pb[:, :csz], copy)
                    if last_h:
                        # ship the final h per-chunk as each evicts, so
                        # only the smallest piece drains after the last
                        # matmul (a [128,w] DMA costs ~2.1us for ANY w --
                        # never split; overlap instead)
                        e = nc.sync if ci % 2 == 0 else nc.scalar
                        e.dma_start(out=outT[h][:, coff:coff + csz],
                                    in_=oh[:, coff:coff + csz])
                if not last_h:
                    e = nc.sync if h % 2 == 0 else nc.scalar
                    e.dma_start(out=outT[h], in_=oh[:])

    nc.compile()
    return nc


def kernel(hidden_states, gate_w, w1, w2, w3, _trace=False):
    global LAST_EXEC_TIME_NS
    _ensure_axon_hooks_stub()
    from concourse.bass_utils import run_bass_kernel_spmd

    x = np.asarray(hidden_states, dtype=np.float32).reshape(-1, H)
    gate_w = np.asarray(gate_w, dtype=np.float32)
    w1 = np.asarray(w1, dtype=np.float32)
    w2 = np.asarray(w2, dtype=np.float32)
    w3 = np.asarray(w3, dtype=np.float32)
    T = x.shape[0]

    # Router (f32, same math as the module): softmax over experts, top-2,
    # renormalized weights.
    logits = x @ gate_w.T
    p = np.exp(logits - logits.max(-1, keepdims=True))
    p /= p.sum(-1, keepdims=True)
    sel = np.argpartition(-p, TOP_K - 1, axis=-1)[:, :TOP_K]
    rw = np.take_along_axis(p, sel, axis=-1)
    rw = rw / rw.sum(-1, keepdims=True)

    idx_e, cv_e = [], []
    for e in range(E):
        hit = sel == e                      # [T, K]
        idx = np.nonzero(hit.any(axis=1))[0]
        w = np.where(hit[idx, 0], rw[idx, 0], rw[idx, 1])
        idx_e.append(idx)
        cv_e.append(w.astype(np.float32))

    # SBUF budget (xe + gT residents) caps the per-run token capacity.
    # Actual data peaks at cmax ~1071; the segment loop only engages for
    # pathologically imbalanced routing.
    CMAX_HW = 1344
    cmax = max(len(i) for i in idx_e)
    n_seg = max(1, -(-cmax // CMAX_HW))
    seg_idx = [np.array_split(idx_e[e], n_seg) for e in range(E)]
    seg_cv = [np.array_split(cv_e[e], n_seg) for e in range(E)]
    C = max(512, max(len(s) for parts in seg_idx for s in parts))

    if C not in _BUILD_CACHE:
        _BUILD_CACHE[C] = _build(C)
    nc = _BUILD_CACHE[C]

    x_bf = x.astype(BF16)
    w_packed = []
    for e in range(E):
        w1pk = np.ascontiguousarray(
            w1[e].astype(BF16).reshape(FP, P, HK, P).transpose(0, 3, 2, 1)
        ).reshape(FP, P, H)
        w3pk = np.ascontiguousarray(
            w3[e].astype(BF16).reshape(FP, P, HK, P).transpose(0, 3, 2, 1)
        ).reshape(FP, P, H)
        w2pk = np.ascontiguousarray(w2[e].T.astype(BF16)).reshape(FP, P, H)
        w_packed.append((w1pk, w3pk, w2pk))

    out = np.zeros((T, H), dtype=np.float32)
    LAST_EXEC_TIME_NS = None
    for seg in range(n_seg):
        in_maps = []
        for e in range(E):
            idx = seg_idx[e][seg]
            n = len(idx)
            xeT = np.zeros((H, C), dtype=BF16)
            xeT[:, :n] = x_bf[idx].T
            w1pk, w3pk, w2pk = w_packed[e]
            # pack [P, HK*C]: partition-major rows holding all HK chunks
            xpk = np.ascontiguousarray(
                xeT.reshape(HK, P, C).transpose(1, 0, 2).reshape(P, HK * C))
            in_maps.append({
                "xe": xpk,
                "w1p": w1pk,
                "w3p": w3pk,
                "w2p": w2pk,
            })
        res = run_bass_kernel_spmd(
            nc, in_maps, core_ids=list(range(N_CORES)), trace=_trace
        )
        if res.exec_time_ns is not None:
            LAST_EXEC_TIME_NS = (LAST_EXEC_TIME_NS or 0) + res.exec_time_ns
        for e in range(E):
            idx = seg_idx[e][seg]
            n = len(idx)
            if n:
                oT = np.asarray(res.results[e]["outT"],
                                dtype=np.float32).reshape(H, C)
                out[idx] += oT[:, :n].T * seg_cv[e][seg][:, None]
    return out.reshape(B, S, H)
